# revision 14
# baseline (speedup 1.0000x reference)
"""Trainium2 Bass kernel for CRF Viterbi decode (nn_CRFLayer).

Problem: emissions [512, 1024, 48] f32, mask [512,1024] (unused by reference),
transitions [48,48], start/end_transitions [48]. Output: best_paths [512, 1024]
int32 (Viterbi argmax decode, jax reference semantics: first-occurrence argmax).

Strategy (8 NeuronCores, pure data parallel over batch, 64 seqs/core):

Forward (per core): 128 partitions = (g, b) with g in {0,1} the j-half group,
b the sequence. Group g computes the 24 next-tags j in [24g, 24g+24).
Per step, 5 DVE ops:
  TT   cand[p, jl, i'] = s_full[p, i'] + trep2[p, jl, i']      (1152/partition)
  TR   pre[p, jl]      = max_i' cand                            (grouped max)
  TT   s_full[p, 0:24] = pre + em                               (em add, in place)
  2x stream_shuffle to exchange the halves across groups.
The g=1 replica of the transitions (trep2) is column-rotated by 24 so that each
group's own j-half lands at columns 0:24 of its s_full rows — this makes the
em-add a single 128-partition op with no per-group copies. s_full rows 0:64
hold the natural tag order and are streamed to DRAM as the score history.

Backtrace: tag_t = argmax_i(s_t[b,i] + T[i, tag_{t+1}]) recomputed per step
from the stored history. The transition-column gather T[:, tag_b] is exact via
two small PE matmuls: tag broadcast (bf16, exact for small integers) ->
transpose against identity -> one-hot compare -> one-hot @ T^T (fp32)
accumulated on top of a PSUM bank preloaded with the hist row (so the add is
free), then DVE max + max_index (first-occurrence argmax, matching
jnp.argmax). Selection matmuls (one-hot/identity operands) are exact in any
PE mode, so all arithmetic replicates the reference's f32 ops bit-exactly and
the integer paths match exactly.

All derived constants (trep2, start2, T^T, identity, iota) are precomputed on
the host in kernel() and passed as extra inputs.
"""

import sys
from contextlib import ExitStack

import numpy as np

sys.path.insert(0, "/opt/trn_rl_repo")

import concourse.bass as bass  # noqa: E402
import concourse.tile as tile  # noqa: E402
from concourse import bacc, mybir  # noqa: E402

F32 = mybir.dt.float32
BF16 = mybir.dt.bfloat16
U16 = mybir.dt.uint16
I32 = mybir.dt.int32

NUM_TAGS = 48
BATCH = 512
SEQ_LEN = 1024
N_CORES = 8
B_LOC = BATCH // N_CORES  # 64 sequences per core
H = NUM_TAGS // 2  # 24
IDENT32 = list(range(32))


def build_nc(
    S: int = SEQ_LEN,
    TB: int = 128,
    B: int = B_LOC,
    T: int = NUM_TAGS,
    GPW: int = 0,
    bf16_mm1: bool = True,
    psum_acc: bool = True,
):
    """Build the per-core Bass program (same program on all cores, SPMD).

    GPW: number of j columns (of each group's 24) computed by the GpSimd
    engine concurrently with the Vector engine's slice of the candidate add.
    Measured ~5ns/elem on HW (vs 1.04 on DVE), so 0 is fastest.
    bf16_mm1: run the tag-transpose matmul in bf16 (tags are small integers,
    exact in bf16; one PE pass instead of fp32's two).
    psum_acc: preload the hist row into the tcol PSUM bank and let the
    gather matmul accumulate, eliminating the separate DVE add.
    """
    assert S % TB == 0
    nblk = S // TB
    P = 2 * B  # 128 partitions
    HV = H - GPW  # DVE's share of the j columns

    nc = bacc.Bacc("TRN2", target_bir_lowering=False, debug=False, num_devices=N_CORES)

    em_d = nc.dram_tensor("emissions", [B, S, T], F32, kind="ExternalInput")
    trep2_d = nc.dram_tensor("trep2", [P, H, T], F32, kind="ExternalInput")
    start2_d = nc.dram_tensor("start2", [P, H], F32, kind="ExternalInput")
    endrow_d = nc.dram_tensor("endrow", [B, T], F32, kind="ExternalInput")
    tt_d = nc.dram_tensor("t_t", [T, T], F32, kind="ExternalInput")
    diag_d = nc.dram_tensor("diag01", [B, B], F32, kind="ExternalInput")
    iotap_d = nc.dram_tensor("iota_p", [T, B], F32, kind="ExternalInput")
    paths_d = nc.dram_tensor("paths", [B, S], I32, kind="ExternalOutput")
    hist_d = nc.dram_tensor("hist", [B, S, T], F32, kind="Internal")

    with tile.TileContext(nc) as tc, ExitStack() as ctx:
        const = ctx.enter_context(tc.tile_pool(name="const", bufs=1))
        emp = ctx.enter_context(tc.tile_pool(name="emp", bufs=2))
        hip = ctx.enter_context(tc.tile_pool(name="hip", bufs=2))
        hrp = ctx.enter_context(tc.tile_pool(name="hrp", bufs=2))
        wrk = ctx.enter_context(tc.tile_pool(name="wrk", bufs=3))
        psum = ctx.enter_context(tc.tile_pool(name="psum", bufs=2, space="PSUM"))

        # ---- constants (all host-precomputed) ---------------------------
        trep2 = const.tile([P, H, T], F32, name="trep2")
        nc.sync.dma_start(trep2[:], trep2_d.ap())
        start2 = const.tile([P, H], F32, name="start2")
        nc.sync.dma_start(start2[:], start2_d.ap())
        end_b = const.tile([B, T], F32, name="end_b")
        nc.sync.dma_start(end_b[:], endrow_d.ap())
        t_t = const.tile([T, T], F32, name="t_t")
        nc.sync.dma_start(t_t[:], tt_d.ap())
        diag01 = const.tile([B, B], F32, name="diag01")
        nc.sync.dma_start(diag01[:], diag_d.ap())
        diag01b = const.tile([B, B], BF16, name="diag01b")
        nc.vector.tensor_copy(diag01b[:], diag01[:])
        iota_p = const.tile([T, B], F32, name="iota_p")
        nc.sync.dma_start(iota_p[:], iotap_d.ap())

        # path8[b, t, 0:8]: max_index writes 8-wide rows; col 0 is the tag
        path8 = const.tile([B, S, 8], U16, name="path8")

        # ---- forward ----------------------------------------------------
        hist_prev = None
        hist_t = None
        for blk in range(nblk):
            em_t = emp.tile([P, TB, H], F32, tag="em")
            nc.sync.dma_start(em_t[0:B], em_d.ap()[:, blk * TB : (blk + 1) * TB, 0:H])
            nc.sync.dma_start(
                em_t[B:P], em_d.ap()[:, blk * TB : (blk + 1) * TB, H:T]
            )
            hist_prev = hist_t
            hist_t = hip.tile([P, TB, T], F32, tag="hist")

            for off in range(TB):
                t = blk * TB + off
                if t == 0:
                    nc.vector.tensor_tensor(
                        hist_t[:, 0, 0:H], start2[:], em_t[:, 0, :],
                        op=mybir.AluOpType.add,
                    )
                else:
                    s_prev = (
                        hist_t[:, off - 1, :] if off > 0
                        else hist_prev[:, TB - 1, :]
                    )
                    cand = wrk.tile([P, H, T], F32, tag="cand")
                    if GPW > 0:
                        nc.gpsimd.tensor_tensor(
                            cand[:, HV:H, :],
                            s_prev.unsqueeze(1).broadcast_to([P, GPW, T]),
                            trep2[:, HV:H, :],
                            op=mybir.AluOpType.add,
                        )
                    nc.vector.tensor_tensor(
                        cand[:, 0:HV, :],
                        s_prev.unsqueeze(1).broadcast_to([P, HV, T]),
                        trep2[:, 0:HV, :],
                        op=mybir.AluOpType.add,
                    )
                    pre = wrk.tile([P, H], F32, tag="pre")
                    nc.vector.tensor_reduce(
                        pre[:], cand[:], axis=mybir.AxisListType.X,
                        op=mybir.AluOpType.max,
                    )
                    nc.vector.tensor_tensor(
                        hist_t[:, off, 0:H], pre[:], em_t[:, off, :],
                        op=mybir.AluOpType.add,
                    )
                # exchange halves: each group's own half sits at cols 0:24
                nc.vector.stream_shuffle(
                    hist_t[0:B, off, H:T], hist_t[B:P, off, 0:H], mask=IDENT32
                )
                nc.vector.stream_shuffle(
                    hist_t[B:P, off, H:T], hist_t[0:B, off, 0:H], mask=IDENT32
                )

            nc.sync.dma_start(
                hist_d.ap()[:, blk * TB : (blk + 1) * TB, :], hist_t[0:B]
            )

        # ---- final argmax ----------------------------------------------
        fin = const.tile([B, T], F32, name="fin")
        nc.vector.tensor_tensor(
            fin[:], hist_t[0:B, TB - 1, :], end_b[:], op=mybir.AluOpType.add
        )
        m8f = const.tile([B, 8], F32, name="m8f")
        nc.vector.max(m8f[:], fin[:])
        nc.vector.max_index(path8[:, S - 1, :], m8f[:], fin[:])

        # ---- backtrace --------------------------------------------------
        for rblk in range(nblk - 1, -1, -1):
            hr = hrp.tile([B, TB, T], F32, tag="hr")
            nc.sync.dma_start(hr[:], hist_d.ap()[:, rblk * TB : (rblk + 1) * TB, :])
            for off in range(TB - 1, -1, -1):
                t = rblk * TB + off
                if t == S - 1:
                    continue
                # one-hot of tag_{t+1}: broadcast-cast, transpose via PE,
                # compare against the partition iota
                wrep = wrk.tile([B, T], BF16 if bf16_mm1 else F32, tag="wrep")
                nc.vector.tensor_copy(
                    wrep[:], path8[:, t + 1, 0:1].broadcast_to([B, T])
                )
                tcol = psum.tile([B, T], F32, tag="tcol")
                if psum_acc:
                    # preload hr into the PSUM bank (hidden under MM1),
                    # let MM2 accumulate on top, and skip the separate add
                    nc.vector.tensor_copy(tcol[:], hr[:, off, :])
                tagb = psum.tile([T, B], F32, tag="tagb")
                nc.tensor.matmul(tagb[:], wrep[:], diag01b[:] if bf16_mm1 else diag01[:])
                oht = wrk.tile([T, B], F32, tag="oht")
                nc.vector.tensor_tensor(
                    oht[:], iota_p[:], tagb[:], op=mybir.AluOpType.is_equal
                )
                if psum_acc:
                    nc.tensor.matmul(
                        tcol[:], oht[:], t_t[:], start=False, stop=True,
                        skip_group_check=True,
                    )
                    m8 = wrk.tile([B, 8], F32, tag="m8")
                    nc.vector.max(m8[:], tcol[:])
                    nc.vector.max_index(path8[:, t, :], m8[:], tcol[:])
                else:
                    nc.tensor.matmul(tcol[:], oht[:], t_t[:])
                    c48 = wrk.tile([B, T], F32, tag="c48")
                    nc.vector.tensor_tensor(
                        c48[:], hr[:, off, :], tcol[:], op=mybir.AluOpType.add
                    )
                    m8 = wrk.tile([B, 8], F32, tag="m8")
                    nc.vector.max(m8[:], c48[:])
                    nc.vector.max_index(path8[:, t, :], m8[:], c48[:])

        # ---- emit paths -------------------------------------------------
        paths_i = const.tile([B, S], I32, name="paths_i")
        nc.vector.tensor_copy(paths_i[:], path8[:, :, 0])
        nc.sync.dma_start(paths_d.ap()[:], paths_i[:])

    nc.compile()
    return nc


def make_derived(transitions, start_transitions, end_transitions):
    """Host-precomputed derived constant tensors (per-core replicated)."""
    T = NUM_TAGS
    B = B_LOC
    Tm = np.ascontiguousarray(transitions, dtype=np.float32)
    # trep2[g*64+b, jl, i']: g0: T[i', jl]; g1: T[(i'+24)%48, 24+jl]
    g0 = Tm.T[0:H, :]  # [jl, i] = T[i, jl]
    rot = np.roll(np.arange(T), -H)  # i' -> (i'+24)%48
    g1 = Tm.T[H:T, :][:, rot]  # [jl, i'] = T[(i'+24)%48, 24+jl]
    trep2 = np.empty((2 * B, H, T), dtype=np.float32)
    trep2[0:B] = g0[None, :, :]
    trep2[B:] = g1[None, :, :]
    start2 = np.empty((2 * B, H), dtype=np.float32)
    start2[0:B] = np.asarray(start_transitions, dtype=np.float32)[None, 0:H]
    start2[B:] = np.asarray(start_transitions, dtype=np.float32)[None, H:T]
    endrow = np.broadcast_to(
        np.asarray(end_transitions, dtype=np.float32)[None, :], (B, T)
    ).copy()
    t_t = np.ascontiguousarray(Tm.T)
    diag01 = np.eye(B, dtype=np.float32)
    iota_p = np.broadcast_to(
        np.arange(T, dtype=np.float32)[:, None], (T, B)
    ).copy()
    return {
        "trep2": trep2,
        "start2": start2,
        "endrow": endrow,
        "t_t": t_t,
        "diag01": diag01,
        "iota_p": iota_p,
    }


def make_in_maps(inputs):
    """Shard full inputs into per-core input maps."""
    emissions = np.ascontiguousarray(np.asarray(inputs["emissions"]), dtype=np.float32)
    derived = make_derived(
        np.asarray(inputs["transitions"]),
        np.asarray(inputs["start_transitions"]),
        np.asarray(inputs["end_transitions"]),
    )
    in_maps = []
    for c in range(N_CORES):
        m = {"emissions": emissions[c * B_LOC : (c + 1) * B_LOC]}
        m.update(derived)
        in_maps.append(m)
    return in_maps


def kernel(emissions, mask, transitions, start_transitions, end_transitions):
    """Full-input entry point: shards batch over 8 cores, runs SPMD, gathers."""
    from concourse.bass_utils import run_bass_kernel_spmd

    nc = build_nc()
    in_maps = make_in_maps(
        {
            "emissions": emissions,
            "transitions": transitions,
            "start_transitions": start_transitions,
            "end_transitions": end_transitions,
        }
    )
    res = run_bass_kernel_spmd(nc, in_maps, list(range(N_CORES)))
    out = np.concatenate([np.asarray(r["paths"]) for r in res.results], axis=0)
    return out.astype(np.int32)


# revision 18
# speedup vs baseline: 1.0005x; 1.0005x over previous
"""Trainium2 Bass kernel for CRF Viterbi decode (nn_CRFLayer).

Problem: emissions [512, 1024, 48] f32, mask [512,1024] (unused by reference),
transitions [48,48], start/end_transitions [48]. Output: best_paths [512, 1024]
int32 (Viterbi argmax decode, jax reference semantics: first-occurrence argmax).

Strategy (8 NeuronCores, pure data parallel over batch, 64 seqs/core):

Forward (per core): 128 partitions = (g, b) with g in {0,1} the j-half group,
b the sequence. Group g computes the 24 next-tags j in [24g, 24g+24).
Per step, 5 DVE ops:
  TT   cand[p, jl, i'] = s_full[p, i'] + trep2[p, jl, i']      (1152/partition)
  TR   pre[p, jl]      = max_i' cand                            (grouped max)
  TT   s_full[p, 0:24] = pre + em                               (em add, in place)
  2x stream_shuffle to exchange the halves across groups.
The g=1 replica of the transitions (trep2) is column-rotated by 24 so that each
group's own j-half lands at columns 0:24 of its s_full rows — this makes the
em-add a single 128-partition op with no per-group copies. s_full rows 0:64
hold the natural tag order and are streamed to DRAM as the score history.

Backtrace: tag_t = argmax_i(s_t[b,i] + T[i, tag_{t+1}]) recomputed per step
from the stored history. The transition-column gather T[:, tag_b] is exact via
two small PE matmuls: tag broadcast (bf16, exact for small integers) ->
transpose against identity -> one-hot compare -> one-hot @ T^T (fp32)
accumulated on top of a PSUM bank preloaded with the hist row (so the add is
free), then DVE max + max_index (first-occurrence argmax, matching
jnp.argmax). Selection matmuls (one-hot/identity operands) are exact in any
PE mode, so all arithmetic replicates the reference's f32 ops bit-exactly and
the integer paths match exactly.

All derived constants (trep2, start2, T^T, identity, iota) are precomputed on
the host in kernel() and passed as extra inputs.
"""

import sys
from contextlib import ExitStack

import numpy as np

sys.path.insert(0, "/opt/trn_rl_repo")

import concourse.bass as bass  # noqa: E402
import concourse.tile as tile  # noqa: E402
from concourse import bacc, mybir  # noqa: E402

F32 = mybir.dt.float32
BF16 = mybir.dt.bfloat16
U16 = mybir.dt.uint16
I32 = mybir.dt.int32

NUM_TAGS = 48
BATCH = 512
SEQ_LEN = 1024
N_CORES = 8
B_LOC = BATCH // N_CORES  # 64 sequences per core
H = NUM_TAGS // 2  # 24
IDENT32 = list(range(32))


def build_nc(
    S: int = SEQ_LEN,
    TB: int = 128,
    B: int = B_LOC,
    T: int = NUM_TAGS,
    GPW: int = 0,
    bf16_mm1: bool = True,
    psum_acc: bool = True,
    use_stt: bool = True,
    use_pool: bool = False,
):
    """Build the per-core Bass program (same program on all cores, SPMD).

    GPW: number of j columns (of each group's 24) computed by the GpSimd
    engine concurrently with the Vector engine's slice of the candidate add.
    Measured ~5ns/elem on HW (vs 1.04 on DVE), so 0 is fastest.
    bf16_mm1: run the tag-transpose matmul in bf16 (tags are small integers,
    exact in bf16; one PE pass instead of fp32's two).
    psum_acc: preload the hist row into the tcol PSUM bank and let the
    gather matmul accumulate, eliminating the separate DVE add.
    """
    assert S % TB == 0
    nblk = S // TB
    P = 2 * B  # 128 partitions
    HV = H - GPW  # DVE's share of the j columns

    nc = bacc.Bacc("TRN2", target_bir_lowering=False, debug=False, num_devices=N_CORES)

    em_d = nc.dram_tensor("emissions", [B, S, T], F32, kind="ExternalInput")
    trep2_d = nc.dram_tensor("trep2", [P, H, T], F32, kind="ExternalInput")
    start2_d = nc.dram_tensor("start2", [P, H], F32, kind="ExternalInput")
    endrow_d = nc.dram_tensor("endrow", [B, T], F32, kind="ExternalInput")
    tt_d = nc.dram_tensor("t_t", [T, T], F32, kind="ExternalInput")
    diag_d = nc.dram_tensor("diag01", [B, B], F32, kind="ExternalInput")
    iotap_d = nc.dram_tensor("iota_p", [T, B], F32, kind="ExternalInput")
    paths_d = nc.dram_tensor("paths", [B, S], I32, kind="ExternalOutput")
    hist_d = nc.dram_tensor("hist", [B, S, T], F32, kind="Internal")

    with tile.TileContext(nc) as tc, ExitStack() as ctx:
        const = ctx.enter_context(tc.tile_pool(name="const", bufs=1))
        emp = ctx.enter_context(tc.tile_pool(name="emp", bufs=2))
        hip = ctx.enter_context(tc.tile_pool(name="hip", bufs=2))
        hrp = ctx.enter_context(tc.tile_pool(name="hrp", bufs=2))
        wrk = ctx.enter_context(tc.tile_pool(name="wrk", bufs=3))
        psum = ctx.enter_context(tc.tile_pool(name="psum", bufs=2, space="PSUM"))

        # ---- constants (all host-precomputed) ---------------------------
        trep2 = const.tile([P, H, T], F32, name="trep2")
        nc.sync.dma_start(trep2[:], trep2_d.ap())
        start2 = const.tile([P, H], F32, name="start2")
        nc.sync.dma_start(start2[:], start2_d.ap())
        end_b = const.tile([B, T], F32, name="end_b")
        nc.sync.dma_start(end_b[:], endrow_d.ap())
        t_t = const.tile([T, T], F32, name="t_t")
        nc.sync.dma_start(t_t[:], tt_d.ap())
        diag01 = const.tile([B, B], F32, name="diag01")
        nc.sync.dma_start(diag01[:], diag_d.ap())
        diag01b = const.tile([B, B], BF16, name="diag01b")
        nc.vector.tensor_copy(diag01b[:], diag01[:])
        iota_p = const.tile([T, B], F32, name="iota_p")
        nc.sync.dma_start(iota_p[:], iotap_d.ap())

        # path8[b, t, 0:8]: max_index writes 8-wide rows; col 0 is the tag
        path8 = const.tile([B, S, 8], U16, name="path8")

        # ---- forward ----------------------------------------------------
        hist_prev = None
        hist_t = None
        for blk in range(nblk):
            em_t = emp.tile([P, TB, H], F32, tag="em")
            nc.sync.dma_start(em_t[0:B], em_d.ap()[:, blk * TB : (blk + 1) * TB, 0:H])
            nc.sync.dma_start(
                em_t[B:P], em_d.ap()[:, blk * TB : (blk + 1) * TB, H:T]
            )
            hist_prev = hist_t
            hist_t = hip.tile([P, TB, T], F32, tag="hist")

            for off in range(TB):
                t = blk * TB + off
                if t == 0:
                    nc.vector.tensor_tensor(
                        hist_t[:, 0, 0:H], start2[:], em_t[:, 0, :],
                        op=mybir.AluOpType.add,
                    )
                else:
                    s_prev = (
                        hist_t[:, off - 1, :] if off > 0
                        else hist_prev[:, TB - 1, :]
                    )
                    cand = wrk.tile([P, H, T], F32, tag="cand")
                    if GPW > 0:
                        nc.gpsimd.tensor_tensor(
                            cand[:, HV:H, :],
                            s_prev.unsqueeze(1).broadcast_to([P, GPW, T]),
                            trep2[:, HV:H, :],
                            op=mybir.AluOpType.add,
                        )
                    if use_stt:
                        # same add expressed as scalar_tensor_tensor:
                        # (trep2 max -FLT_MAX) add s  — max(x,-FLT_MAX)=x exactly
                        nc.vector.scalar_tensor_tensor(
                            cand[:, 0:HV, :],
                            trep2[:, 0:HV, :],
                            -3.4028234663852886e38,
                            s_prev.unsqueeze(1).broadcast_to([P, HV, T]),
                            op0=mybir.AluOpType.max,
                            op1=mybir.AluOpType.add,
                        )
                    else:
                        nc.vector.tensor_tensor(
                            cand[:, 0:HV, :],
                            s_prev.unsqueeze(1).broadcast_to([P, HV, T]),
                            trep2[:, 0:HV, :],
                            op=mybir.AluOpType.add,
                        )
                    pre = wrk.tile([P, H], F32, tag="pre")
                    if use_pool:
                        nc.vector.pool_max(pre[:], cand[:])
                    else:
                        nc.vector.tensor_reduce(
                            pre[:], cand[:], axis=mybir.AxisListType.X,
                            op=mybir.AluOpType.max,
                        )
                    nc.vector.tensor_tensor(
                        hist_t[:, off, 0:H], pre[:], em_t[:, off, :],
                        op=mybir.AluOpType.add,
                    )
                # exchange halves: each group's own half sits at cols 0:24
                nc.vector.stream_shuffle(
                    hist_t[0:B, off, H:T], hist_t[B:P, off, 0:H], mask=IDENT32
                )
                nc.vector.stream_shuffle(
                    hist_t[B:P, off, H:T], hist_t[0:B, off, 0:H], mask=IDENT32
                )

            nc.sync.dma_start(
                hist_d.ap()[:, blk * TB : (blk + 1) * TB, :], hist_t[0:B]
            )

        # ---- final argmax ----------------------------------------------
        fin = const.tile([B, T], F32, name="fin")
        nc.vector.tensor_tensor(
            fin[:], hist_t[0:B, TB - 1, :], end_b[:], op=mybir.AluOpType.add
        )
        m8f = const.tile([B, 8], F32, name="m8f")
        nc.vector.max(m8f[:], fin[:])
        nc.vector.max_index(path8[:, S - 1, :], m8f[:], fin[:])

        # ---- backtrace --------------------------------------------------
        for rblk in range(nblk - 1, -1, -1):
            hr = hrp.tile([B, TB, T], F32, tag="hr")
            nc.sync.dma_start(hr[:], hist_d.ap()[:, rblk * TB : (rblk + 1) * TB, :])
            for off in range(TB - 1, -1, -1):
                t = rblk * TB + off
                if t == S - 1:
                    continue
                # one-hot of tag_{t+1}: broadcast-cast, transpose via PE,
                # compare against the partition iota
                wrep = wrk.tile([B, T], BF16 if bf16_mm1 else F32, tag="wrep")
                nc.vector.tensor_copy(
                    wrep[:], path8[:, t + 1, 0:1].broadcast_to([B, T])
                )
                tcol = psum.tile([B, T], F32, tag="tcol")
                if psum_acc:
                    # preload hr into the PSUM bank (hidden under MM1),
                    # let MM2 accumulate on top, and skip the separate add
                    nc.vector.tensor_copy(tcol[:], hr[:, off, :])
                tagb = psum.tile([T, B], F32, tag="tagb")
                nc.tensor.matmul(tagb[:], wrep[:], diag01b[:] if bf16_mm1 else diag01[:])
                oht = wrk.tile([T, B], F32, tag="oht")
                nc.vector.tensor_tensor(
                    oht[:], iota_p[:], tagb[:], op=mybir.AluOpType.is_equal
                )
                if psum_acc:
                    nc.tensor.matmul(
                        tcol[:], oht[:], t_t[:], start=False, stop=True,
                        skip_group_check=True,
                    )
                    m8 = wrk.tile([B, 8], F32, tag="m8")
                    nc.vector.max(m8[:], tcol[:])
                    nc.vector.max_index(path8[:, t, :], m8[:], tcol[:])
                else:
                    nc.tensor.matmul(tcol[:], oht[:], t_t[:])
                    c48 = wrk.tile([B, T], F32, tag="c48")
                    nc.vector.tensor_tensor(
                        c48[:], hr[:, off, :], tcol[:], op=mybir.AluOpType.add
                    )
                    m8 = wrk.tile([B, 8], F32, tag="m8")
                    nc.vector.max(m8[:], c48[:])
                    nc.vector.max_index(path8[:, t, :], m8[:], c48[:])

        # ---- emit paths -------------------------------------------------
        paths_i = const.tile([B, S], I32, name="paths_i")
        nc.vector.tensor_copy(paths_i[:], path8[:, :, 0])
        nc.sync.dma_start(paths_d.ap()[:], paths_i[:])

    nc.compile()
    return nc


def make_derived(transitions, start_transitions, end_transitions):
    """Host-precomputed derived constant tensors (per-core replicated)."""
    T = NUM_TAGS
    B = B_LOC
    Tm = np.ascontiguousarray(transitions, dtype=np.float32)
    # trep2[g*64+b, jl, i']: g0: T[i', jl]; g1: T[(i'+24)%48, 24+jl]
    g0 = Tm.T[0:H, :]  # [jl, i] = T[i, jl]
    rot = np.roll(np.arange(T), -H)  # i' -> (i'+24)%48
    g1 = Tm.T[H:T, :][:, rot]  # [jl, i'] = T[(i'+24)%48, 24+jl]
    trep2 = np.empty((2 * B, H, T), dtype=np.float32)
    trep2[0:B] = g0[None, :, :]
    trep2[B:] = g1[None, :, :]
    start2 = np.empty((2 * B, H), dtype=np.float32)
    start2[0:B] = np.asarray(start_transitions, dtype=np.float32)[None, 0:H]
    start2[B:] = np.asarray(start_transitions, dtype=np.float32)[None, H:T]
    endrow = np.broadcast_to(
        np.asarray(end_transitions, dtype=np.float32)[None, :], (B, T)
    ).copy()
    t_t = np.ascontiguousarray(Tm.T)
    diag01 = np.eye(B, dtype=np.float32)
    iota_p = np.broadcast_to(
        np.arange(T, dtype=np.float32)[:, None], (T, B)
    ).copy()
    return {
        "trep2": trep2,
        "start2": start2,
        "endrow": endrow,
        "t_t": t_t,
        "diag01": diag01,
        "iota_p": iota_p,
    }


def make_in_maps(inputs):
    """Shard full inputs into per-core input maps."""
    emissions = np.ascontiguousarray(np.asarray(inputs["emissions"]), dtype=np.float32)
    derived = make_derived(
        np.asarray(inputs["transitions"]),
        np.asarray(inputs["start_transitions"]),
        np.asarray(inputs["end_transitions"]),
    )
    in_maps = []
    for c in range(N_CORES):
        m = {"emissions": emissions[c * B_LOC : (c + 1) * B_LOC]}
        m.update(derived)
        in_maps.append(m)
    return in_maps


def kernel(emissions, mask, transitions, start_transitions, end_transitions):
    """Full-input entry point: shards batch over 8 cores, runs SPMD, gathers."""
    from concourse.bass_utils import run_bass_kernel_spmd

    nc = build_nc()
    in_maps = make_in_maps(
        {
            "emissions": emissions,
            "transitions": transitions,
            "start_transitions": start_transitions,
            "end_transitions": end_transitions,
        }
    )
    res = run_bass_kernel_spmd(nc, in_maps, list(range(N_CORES)))
    out = np.concatenate([np.asarray(r["paths"]) for r in res.results], axis=0)
    return out.astype(np.int32)
